# revision 10
# baseline (speedup 1.0000x reference)
"""Trainium2 Bass kernel for e3nn-style GNN message passing convolution.

Strategy (8 cores, no collectives):
 - Shard edges by DESTINATION node range: core k owns nodes [1250k, 1250(k+1))
   and all edges pointing into that range; host concatenates core outputs.
 - Within a core, nodes are bin-packed into 128-node destination blocks
   (balancing per-block edge counts across cores to minimize tile padding).
   Per block, a 4-bank PSUM tile [128 nodes, 2048] accumulates Sel.T @ t
   scatter matmuls over the block's edge tiles, where t[e, (c,u)] are the
   UNREDUCED tensor-product terms: the u-contraction of the TP is absorbed
   into a cheap per-block reduce after the scatter.
 - All GEMMs run in plain bf16. Path norm constants are folded into W2
   columns host-side. The ss|vv half of the per-edge weights is consumed
   directly from PSUM by DVE (no ScalarE drain); only sv|vs is drained.
 - Source-node features are gathered host-side (pure data movement) and
   streamed as a dense per-edge array; z-vectors are built on-chip with
   group-batched ops spread across Pool/DVE.
"""
import math
from contextlib import ExitStack

import numpy as np

import concourse.bass as bass
import concourse.tile as tile
from concourse import bacc, mybir
from concourse import bass_utils

N_NODES = 10000
N_EDGES = 160000
MUL = 16
DIM_EMB = 64
HID = 256
NCORES = 8
NPC = N_NODES // NCORES          # 1250 nodes per core
P = 128
NBLK = math.ceil(NPC / P)        # 10 node blocks per core
C3 = 1.0 / math.sqrt(3.0)
ALPHA = 1.0 / math.sqrt(2 * MUL)
GN = 1.0 / math.sqrt(N_EDGES / N_NODES)   # segment-sum normalization
AGN = ALPHA * GN
ACGN = ALPHA * C3 * GN

F32 = mybir.dt.float32
BF16 = mybir.dt.bfloat16

_CACHE = {}


def _build(tiles_per_block):
    """Build the Bass program for a fixed per-block tile schedule."""
    T = sum(tiles_per_block)           # total 128-edge tiles per core
    NG = T // 4                        # 512-edge groups (T padded to 4)
    nc = bacc.Bacc("TRN2", target_bir_lowering=False, debug=False,
                   num_devices=NCORES)

    embT_d = nc.dram_tensor("embT", [64, T * P], BF16, kind="ExternalInput").ap()
    srcp_d = nc.dram_tensor("srcp", [NG * P, 256], BF16, kind="ExternalInput").ap()
    attrp_d = nc.dram_tensor("attrp", [NG * P, 16], F32, kind="ExternalInput").ap()
    dstr_d = nc.dram_tensor("dstr", [NG * P, 4], BF16, kind="ExternalInput").ap()
    w1_d = nc.dram_tensor("w1", [64, HID], BF16, kind="ExternalInput").ap()
    b1_d = nc.dram_tensor("b1", [HID, 1], F32, kind="ExternalInput").ap()
    w2_d = nc.dram_tensor("w2", [HID, 1024], BF16, kind="ExternalInput").ap()
    iota_d = nc.dram_tensor("iota", [P, P], BF16, kind="ExternalInput").ap()
    out_d = nc.dram_tensor("out", [NBLK * P, 64], F32, kind="ExternalOutput").ap()

    # block id for each tile, and first/last flags
    tile_blk, first, last = [], [], []
    for b, nt in enumerate(tiles_per_block):
        for i in range(nt):
            tile_blk.append(b)
            first.append(i == 0)
            last.append(i == nt - 1)

    with tile.TileContext(nc) as tc, ExitStack() as ctx:
        const = ctx.enter_context(tc.tile_pool(name="const", bufs=1))
        sbB = ctx.enter_context(tc.tile_pool(name="sbB", bufs=3))   # group streams
        sbT = ctx.enter_context(tc.tile_pool(name="sbT", bufs=3))   # per-subtile
        sbO = ctx.enter_context(tc.tile_pool(name="sbO", bufs=2))   # block out
        psH = ctx.enter_context(tc.tile_pool(name="psH", bufs=2, space="PSUM"))
        psW = ctx.enter_context(tc.tile_pool(name="psW", bufs=2, space="PSUM"))
        psO = ctx.enter_context(tc.tile_pool(name="psO", bufs=1, space="PSUM"))

        # constants
        w1_t = const.tile([64, HID], BF16)
        nc.sync.dma_start(w1_t[:], w1_d[:])
        b1_h = [const.tile([P, 1], F32, name=f"b1_{i}", tag=f"b1_{i}") for i in range(2)]
        w2_h = [const.tile([P, 1024], BF16, name=f"w2_{i}", tag=f"w2_{i}") for i in range(2)]
        for i in range(2):
            nc.sync.dma_start(b1_h[i][:], b1_d[i * P:(i + 1) * P, :])
            nc.sync.dma_start(w2_h[i][:], w2_d[i * P:(i + 1) * P, :])
        iota_t = const.tile([P, P], BF16)
        nc.sync.dma_start(iota_t[:], iota_d[:])

        sc_ps = None
        cur_blk = -1
        for g in range(NG):
            e0 = g * 512
            # ---- group loads
            embT = sbB.tile([64, 512], BF16, tag="embT")
            nc.sync.dma_start(embT[:], embT_d[:, e0:e0 + 512])
            srcp = sbB.tile([P, 256], BF16, tag="srcp")
            nc.sync.dma_start(srcp[:], srcp_d[g * P:(g + 1) * P, :])
            attrp = sbB.tile([P, 16], F32, tag="attrp")
            nc.sync.dma_start(attrp[:], attrp_d[g * P:(g + 1) * P, :])
            dst4 = sbB.tile([P, 4], BF16, tag="dst4")
            nc.sync.dma_start(dst4[:], dstr_d[g * P:(g + 1) * P, :])

            # ---- W1 GEMM + silu -> gT [h, e] bf16
            gT = [sbB.tile([P, 512], BF16, name=f"gT{hh}", tag=f"gT{hh}")
                  for hh in range(2)]
            for hh in range(2):
                h_ps = psH.tile([P, 512], F32, space="PSUM", tag="hps")
                nc.tensor.matmul(h_ps[:], w1_t[:, hh * P:(hh + 1) * P], embT[:],
                                 start=True, stop=True)
                nc.scalar.activation(gT[hh][:], h_ps[:],
                                     mybir.ActivationFunctionType.Silu,
                                     bias=b1_h[hh][:])

            # ---- group-batched z-builds (constants folded into W2 host-side)
            # attrp cols per subtile s (stride 4): [s2, v2x, v2y, v2z]
            attr4 = attrp[:].rearrange("p (s c) -> p s c", c=4)
            # sel4 [128, (s,n)] bf16 = is_equal(dstr, iota)
            sel4 = sbB.tile([P, 512], BF16, tag="sel4")
            nc.vector.tensor_tensor(
                out=sel4[:].rearrange("p (s n) -> p s n", n=P),
                in0=dst4[:][:, :, None].to_broadcast([P, 4, P]),
                in1=iota_t[:, None, :].to_broadcast([P, 4, P]),
                op=mybir.AluOpType.is_equal)

            # Z4 [128, (s,128)] bf16: per subtile [zss 16 | zvv 16 | z_sv(m,u) 48
            #                                      | z_vs(m,u) 48]
            Z4 = sbB.tile([P, 512], BF16, tag="Z4")
            Z4v = Z4[:].rearrange("p (s c) -> p s c", c=128)
            src4 = srcp[:].rearrange("p (s c) -> p s c", c=64)
            s1v = src4[:, :, 0:16]                              # [p, s, u]
            # zss = s1 * s2
            nc.gpsimd.tensor_tensor(
                out=Z4v[:, :, 0:16], in0=s1v,
                in1=attr4[:, :, 0:1].to_broadcast([P, 4, 16]),
                op=mybir.AluOpType.mult)
            # z_vs (zm) = v1[(u,m)->(m,u)] * s2
            nc.gpsimd.tensor_tensor(
                out=Z4v[:, :, 80:128].rearrange("p s (m u) -> p s m u", u=16),
                in0=src4[:, :, 16:64].rearrange("p s (u m) -> p s m u", m=3),
                in1=attr4[:, :, 0:1][:, :, :, None].to_broadcast([P, 4, 3, 16]),
                op=mybir.AluOpType.mult)
            # z_sv = s1 (x) v2
            nc.gpsimd.tensor_tensor(
                out=Z4v[:, :, 32:80].rearrange("p s (m u) -> p s m u", u=16),
                in0=s1v[:, :, None, :].to_broadcast([P, 4, 3, 16]),
                in1=attr4[:, :, 1:4][:, :, :, None].to_broadcast([P, 4, 3, 16]),
                op=mybir.AluOpType.mult)
            # pvv = v1 * v2 (u,m); zvv = sum_m (bf16 accumulate is fine here:
            # 3-term sum feeding a bf16 product path)
            pvv = sbB.tile([P, 192], F32, tag="pvv")
            nc.gpsimd.tensor_tensor(
                out=pvv[:].rearrange("p (s u m) -> p s u m", s=4, m=3),
                in0=src4[:, :, 16:64].rearrange("p s (u m) -> p s u m", m=3),
                in1=attr4[:, :, 1:4][:, :, None, :].to_broadcast([P, 4, 16, 3]),
                op=mybir.AluOpType.mult)
            with nc.allow_low_precision("zvv: 3-term bf16 sum within tolerance"):
                nc.vector.reduce_sum(
                    Z4v[:, :, 16:32][:, :, :, None],
                    pvv[:].rearrange("p (s u m) -> p s u m", s=4, m=3),
                    axis=mybir.AxisListType.X)

            for s in range(4):
                t_idx = g * 4 + s
                blk = tile_blk[t_idx]

                # ---- W2 GEMM. Host col layout: [0:512]=[sv|vs] (drained),
                # [512:1024]=[ss|vv] (kept in PSUM, read by DVE directly).
                psA = psW.tile([P, 512], F32, space="PSUM", tag="wps")
                for kk in range(2):
                    nc.tensor.matmul(psA[:], gT[kk][:, s * P:(s + 1) * P],
                                     w2_h[kk][:, 512:1024],
                                     start=(kk == 0), stop=(kk == 1))
                psB = psW.tile([P, 512], F32, space="PSUM", tag="wps")
                for kk in range(2):
                    nc.tensor.matmul(psB[:], gT[kk][:, s * P:(s + 1) * P],
                                     w2_h[kk][:, 0:512],
                                     start=(kk == 0), stop=(kk == 1))
                wbB = sbT.tile([P, 512], BF16, tag="wbB")
                nc.scalar.activation(wbB[:], psB[:],
                                     mybir.ActivationFunctionType.Copy)

                # ---- products -> t [128, 2048] bf16, cols = (c, u) u-inner
                # c = [ss_v 16 | vv_v 16 | sv (v,m) 48 | vs (v,m) 48]
                t_t = sbT.tile([P, 2048], BF16, tag="t_t")
                Zs = Z4[:, s * 128:(s + 1) * 128]
                # ss+vv from PSUM chunk A
                nc.vector.tensor_mul(
                    t_t[:, 0:512].rearrange("p (g v u) -> p g v u", g=2, u=16),
                    psA[:].rearrange("p (g v u) -> p g v u", g=2, u=16),
                    Zs[:, 0:32].rearrange("p (g u) -> p g u", g=2)[:, :, None, :]
                        .to_broadcast([P, 2, 16, 16]))
                # sv: t[(v,m,u)] = wbB_sv[(v,u)] * z_sv[(m,u)]   (DVE)
                nc.vector.tensor_mul(
                    t_t[:, 512:1280].rearrange("p (v m u) -> p v m u", m=3, u=16),
                    wbB[:, None, 0:256].rearrange("p x (v u) -> p v x u", u=16)
                        .to_broadcast([P, 16, 3, 16]),
                    Zs[:, None, 32:80].rearrange("p x (m u) -> p x m u", u=16)
                        .to_broadcast([P, 16, 3, 16]))
                # vs: t[(v,m,u)] = wbB_vs[(v,u)] * z_vs[(m,u)]   (Pool)
                nc.gpsimd.tensor_mul(
                    t_t[:, 1280:2048].rearrange("p (v m u) -> p v m u", m=3, u=16),
                    wbB[:, None, 256:512].rearrange("p x (v u) -> p v x u", u=16)
                        .to_broadcast([P, 16, 3, 16]),
                    Zs[:, None, 80:128].rearrange("p x (m u) -> p x m u", u=16)
                        .to_broadcast([P, 16, 3, 16]))

                # ---- scatter: psO += Sel.T @ t  (4 x 512-col matmuls)
                if first[t_idx]:
                    sc_ps = psO.tile([P, 2048], F32, space="PSUM", tag="scps")
                    cur_blk = blk
                assert cur_blk == blk
                sel_s = sel4[:, s * P:(s + 1) * P]
                for c5 in range(4):
                    nc.tensor.matmul(sc_ps[:, c5 * 512:(c5 + 1) * 512],
                                     sel_s,
                                     t_t[:, c5 * 512:(c5 + 1) * 512],
                                     start=first[t_idx], stop=last[t_idx])

                if last[t_idx]:
                    # drain psO -> bf16, reduce over u, assemble, store
                    acc = sbO.tile([P, 2048], BF16, tag="acc")
                    nc.scalar.activation(acc[:, 0:1024], sc_ps[:, 0:1024],
                                         mybir.ActivationFunctionType.Copy)
                    nc.vector.tensor_copy(acc[:, 1024:2048], sc_ps[:, 1024:2048])
                    red = sbO.tile([P, 128], F32, tag="red")
                    nc.vector.reduce_sum(
                        red[:][:, :, None],
                        acc[:].rearrange("p (c u) -> p c u", u=16),
                        axis=mybir.AxisListType.X)
                    outb = sbO.tile([P, 64], F32, tag="outb")
                    nc.vector.tensor_add(outb[:, 0:16], red[:, 0:16], red[:, 16:32])
                    nc.vector.tensor_add(outb[:, 16:64], red[:, 32:80],
                                         red[:, 80:128])
                    nc.sync.dma_start(out_d[blk * P:(blk + 1) * P, :], outb[:])

    nc.finalize()
    return nc


def _balance_blocks(edst):
    """Per core: bin-pack nodes into NBLK blocks of <=128 nodes, balancing
    per-block edge counts. Returns row[node] (position in the padded
    [NBLK*128] output) for every global node id."""
    row = np.empty(N_NODES, np.int64)
    deg_all = np.bincount(edst, minlength=N_NODES)
    for k in range(NCORES):
        nodes = np.arange(k * NPC, (k + 1) * NPC)
        deg = deg_all[nodes]
        order = np.argsort(-deg, kind="stable")
        sums = np.zeros(NBLK, np.int64)
        fill = np.zeros(NBLK, np.int64)
        for n in order:
            cand = np.where(fill < P)[0]
            b = cand[np.argmin(sums[cand])]
            row[nodes[n]] = b * P + fill[b]
            fill[b] += 1
            sums[b] += deg[n]
    return row


def _prep(inputs):
    """Host-side sharding: bucket edges by (core, node-block), pad to a
    uniform per-block tile grid, gather source features, build per-core
    input maps. Pure data movement + dtype casts (no value arithmetic)."""
    nf = np.ascontiguousarray(inputs["node_features"], dtype=np.float32)
    esrc = inputs["edge_src"].astype(np.int64)
    edst = inputs["edge_dst"].astype(np.int64)
    eattr = np.asarray(inputs["edge_attr"], dtype=np.float32)
    eemb = np.asarray(inputs["edge_embedding"], dtype=np.float32)

    row = _balance_blocks(edst)
    core = edst // NPC
    erow = row[edst]                      # padded row of the dst node
    blk = erow // P
    key = core * NBLK + blk
    order = np.argsort(key, kind="stable")
    key_s = key[order]

    counts = np.bincount(key_s, minlength=NCORES * NBLK).reshape(NCORES, NBLK)
    tiles_per_block = [int(math.ceil(max(counts[:, b].max(), 1) / P))
                       for b in range(NBLK)]
    while sum(tiles_per_block) % 4:
        tiles_per_block[-1] += 1
    T = sum(tiles_per_block)
    E = T * P
    NG = T // 4
    starts = np.concatenate([[0], np.cumsum([t * P for t in tiles_per_block])])

    import ml_dtypes
    bf16 = ml_dtypes.bfloat16

    in_maps = []
    boundaries = np.searchsorted(key_s, np.arange(NCORES * NBLK + 1))
    for k in range(NCORES):
        emb_full = np.zeros((E, 64), np.float32)
        src_full = np.zeros((E, 64), np.float32)
        attr_full = np.zeros((E, 4), np.float32)
        dst_full = np.full((E,), 999.0, np.float32)
        for b in range(NBLK):
            lo, hi = boundaries[k * NBLK + b], boundaries[k * NBLK + b + 1]
            n = hi - lo
            sl = order[lo:hi]
            o = int(starts[b])
            emb_full[o:o + n] = eemb[sl]
            src_full[o:o + n] = nf[esrc[sl]]
            attr_full[o:o + n] = eattr[sl]
            dst_full[o:o + n] = (erow[sl] - b * P).astype(np.float32)
        embT = np.ascontiguousarray(emb_full.T.astype(bf16))
        srcp = np.ascontiguousarray(
            src_full.reshape(NG, 4, P, 64).transpose(0, 2, 1, 3)
            .reshape(NG * P, 256).astype(bf16))
        attrp = np.ascontiguousarray(
            attr_full.reshape(NG, 4, P, 4).transpose(0, 2, 1, 3)
            .reshape(NG * P, 16))
        dstr = np.ascontiguousarray(
            dst_full.reshape(NG, 4, P).transpose(0, 2, 1)
            .reshape(NG * P, 4).astype(bf16))
        in_maps.append(dict(embT=embT, srcp=srcp, attrp=attrp, dstr=dstr))
    return tiles_per_block, in_maps, row


def _w2_scaled_perm():
    """W2 cols permuted to [sv|vs|ss|vv], each (v,u) u-inner, with path norm
    constants folded in. Returns (perm, scales)."""
    idx = np.arange(1024).reshape(4, MUL, MUL)  # (path, u, v): ss, vs, sv, vv
    blocks = [idx[2].T, idx[1].T, idx[0].T, idx[3].T]  # sv, vs, ss, vv as (v,u)
    perm = np.concatenate([b.reshape(-1) for b in blocks])
    scales = np.concatenate([
        np.full(256, ACGN, np.float32),   # sv
        np.full(256, ACGN, np.float32),   # vs
        np.full(256, AGN, np.float32),    # ss
        np.full(256, ACGN, np.float32),   # vv
    ])
    return perm, scales


def _full_maps(inputs):
    import ml_dtypes
    bf16 = ml_dtypes.bfloat16
    tiles_per_block, in_maps, row = _prep(inputs)
    perm, scales = _w2_scaled_perm()
    W1 = np.ascontiguousarray(np.asarray(inputs["W1"], np.float32).astype(bf16))
    b1 = np.ascontiguousarray(inputs["b1"], np.float32).reshape(HID, 1)
    W2p = np.ascontiguousarray(
        (np.asarray(inputs["W2"], np.float32)[:, perm] * scales[None, :])
        .astype(bf16))
    iota = np.tile(np.arange(P, dtype=np.float32)[None, :], (P, 1)).astype(bf16)
    assert not np.any(inputs["b2"]), "b2 != 0 unsupported by this build"
    for m in in_maps:
        m.update(w1=W1, b1=b1, w2=W2p, iota=iota)
    return tiles_per_block, in_maps, row


def kernel(**inputs):
    tiles_per_block, in_maps, row = _full_maps(inputs)
    key = tuple(tiles_per_block)
    if key not in _CACHE:
        _CACHE[key] = _build(tiles_per_block)
    nc = _CACHE[key]

    res = bass_utils.run_bass_kernel_spmd(nc, in_maps,
                                          core_ids=list(range(NCORES)))
    out = np.empty((N_NODES, 64), np.float32)
    for k in range(NCORES):
        nodes = np.arange(k * NPC, (k + 1) * NPC)
        out[nodes] = res.results[k]["out"][row[nodes]]
    return out


# revision 12
# speedup vs baseline: 1.2887x; 1.2887x over previous
"""Trainium2 Bass kernel for e3nn-style GNN message passing convolution.

Strategy (8 cores, no collectives):
 - Shard edges by DESTINATION node range: core k owns nodes [1250k, 1250(k+1))
   and all edges pointing into that range; host concatenates core outputs.
 - Within a core, nodes are bin-packed into 128-node destination blocks
   (balancing per-block edge counts across cores to minimize tile padding).
   Per block, a 4-bank PSUM tile [128 nodes, 2048] accumulates Sel.T @ t
   scatter matmuls over the block's edge tiles, where t[e, (c,u)] are the
   UNREDUCED tensor-product terms: the u-contraction of the TP is absorbed
   into a cheap per-block reduce after the scatter.
 - All GEMMs run in plain bf16. Path norm constants are folded into W2
   columns host-side. The ss|vv half of the per-edge weights is consumed
   directly from PSUM by DVE (no ScalarE drain); only sv|vs is drained.
 - Source-node features are gathered host-side (pure data movement) and
   streamed as a dense per-edge array; z-vectors are built on-chip with
   group-batched ops spread across Pool/DVE.
"""
import math
from contextlib import ExitStack

import numpy as np

import concourse.bass as bass
import concourse.tile as tile
from concourse import bacc, mybir
from concourse import bass_utils

N_NODES = 10000
N_EDGES = 160000
MUL = 16
DIM_EMB = 64
HID = 256
NCORES = 8
NPC = N_NODES // NCORES          # 1250 nodes per core
P = 128
NBLK = math.ceil(NPC / P)        # 10 node blocks per core
C3 = 1.0 / math.sqrt(3.0)
ALPHA = 1.0 / math.sqrt(2 * MUL)
GN = 1.0 / math.sqrt(N_EDGES / N_NODES)   # segment-sum normalization
AGN = ALPHA * GN
ACGN = ALPHA * C3 * GN

F32 = mybir.dt.float32
BF16 = mybir.dt.bfloat16

_CACHE = {}


def _build(tiles_per_block):
    """Build the Bass program for a fixed per-block tile schedule."""
    T = sum(tiles_per_block)           # total 128-edge tiles per core
    NG = T // 4                        # 512-edge groups (T padded to 4)
    nc = bacc.Bacc("TRN2", target_bir_lowering=False, debug=False,
                   num_devices=NCORES)

    embT_d = nc.dram_tensor("embT", [64, T * P], BF16, kind="ExternalInput").ap()
    srcp_d = nc.dram_tensor("srcp", [NG * P, 256], BF16, kind="ExternalInput").ap()
    attrp_d = nc.dram_tensor("attrp", [NG * P, 16], F32, kind="ExternalInput").ap()
    dstr_d = nc.dram_tensor("dstr", [NG * P, 4], BF16, kind="ExternalInput").ap()
    w1_d = nc.dram_tensor("w1", [64, HID], BF16, kind="ExternalInput").ap()
    b1_d = nc.dram_tensor("b1", [HID, 1], F32, kind="ExternalInput").ap()
    w2_d = nc.dram_tensor("w2", [HID, 1024], BF16, kind="ExternalInput").ap()
    iota_d = nc.dram_tensor("iota", [P, P], BF16, kind="ExternalInput").ap()
    out_d = nc.dram_tensor("out", [NBLK * P, 64], F32, kind="ExternalOutput").ap()

    # block id for each tile, and first/last flags
    tile_blk, first, last = [], [], []
    for b, nt in enumerate(tiles_per_block):
        for i in range(nt):
            tile_blk.append(b)
            first.append(i == 0)
            last.append(i == nt - 1)

    with tile.TileContext(nc) as tc, ExitStack() as ctx:
        const = ctx.enter_context(tc.tile_pool(name="const", bufs=1))
        sbB = ctx.enter_context(tc.tile_pool(name="sbB", bufs=3))   # group streams
        sbT = ctx.enter_context(tc.tile_pool(name="sbT", bufs=3))   # per-subtile
        sbO = ctx.enter_context(tc.tile_pool(name="sbO", bufs=2))   # block out
        psH = ctx.enter_context(tc.tile_pool(name="psH", bufs=2, space="PSUM"))
        psW = ctx.enter_context(tc.tile_pool(name="psW", bufs=2, space="PSUM"))
        psO = ctx.enter_context(tc.tile_pool(name="psO", bufs=1, space="PSUM"))

        # constants
        w1_t = const.tile([64, HID], BF16)
        nc.sync.dma_start(w1_t[:], w1_d[:])
        b1_h = [const.tile([P, 1], F32, name=f"b1_{i}", tag=f"b1_{i}") for i in range(2)]
        w2_h = [const.tile([P, 1024], BF16, name=f"w2_{i}", tag=f"w2_{i}") for i in range(2)]
        for i in range(2):
            nc.sync.dma_start(b1_h[i][:], b1_d[i * P:(i + 1) * P, :])
            nc.sync.dma_start(w2_h[i][:], w2_d[i * P:(i + 1) * P, :])
        iota_t = const.tile([P, P], BF16)
        nc.sync.dma_start(iota_t[:], iota_d[:])

        sc_ps = None
        cur_blk = -1
        for g in range(NG):
            e0 = g * 512
            # ---- group loads
            embT = sbB.tile([64, 512], BF16, tag="embT")
            nc.sync.dma_start(embT[:], embT_d[:, e0:e0 + 512])
            srcp = sbB.tile([P, 256], BF16, tag="srcp")
            nc.sync.dma_start(srcp[:], srcp_d[g * P:(g + 1) * P, :])
            attrp = sbB.tile([P, 16], F32, tag="attrp")
            nc.sync.dma_start(attrp[:], attrp_d[g * P:(g + 1) * P, :])
            dst4 = sbB.tile([P, 4], BF16, tag="dst4")
            nc.sync.dma_start(dst4[:], dstr_d[g * P:(g + 1) * P, :])

            # ---- W1 GEMM + silu -> gT [h, e] bf16
            gT = [sbB.tile([P, 512], BF16, name=f"gT{hh}", tag=f"gT{hh}")
                  for hh in range(2)]
            for hh in range(2):
                h_ps = psH.tile([P, 512], F32, space="PSUM", tag="hps")
                nc.tensor.matmul(h_ps[:], w1_t[:, hh * P:(hh + 1) * P], embT[:],
                                 start=True, stop=True)
                nc.scalar.activation(gT[hh][:], h_ps[:],
                                     mybir.ActivationFunctionType.Silu,
                                     bias=b1_h[hh][:])

            # ---- group-batched z-builds (constants folded into W2 host-side)
            # attrp cols per subtile s (stride 4): [s2, v2x, v2y, v2z]
            attr4 = attrp[:].rearrange("p (s c) -> p s c", c=4)
            # sel4 [128, (s,n)] bf16 = is_equal(dstr, iota)
            sel4 = sbB.tile([P, 512], BF16, tag="sel4")
            nc.vector.tensor_tensor(
                out=sel4[:].rearrange("p (s n) -> p s n", n=P),
                in0=dst4[:][:, :, None].to_broadcast([P, 4, P]),
                in1=iota_t[:, None, :].to_broadcast([P, 4, P]),
                op=mybir.AluOpType.is_equal)

            # Z4 [128, (s,128)] bf16: per subtile [zss 16 | zvv 16 | z_sv(m,u) 48
            #                                      | z_vs(m,u) 48]
            Z4 = sbB.tile([P, 512], BF16, tag="Z4")
            Z4v = Z4[:].rearrange("p (s c) -> p s c", c=128)
            src4 = srcp[:].rearrange("p (s c) -> p s c", c=64)
            s1v = src4[:, :, 0:16]                              # [p, s, u]
            # zss = s1 * s2
            nc.gpsimd.tensor_tensor(
                out=Z4v[:, :, 0:16], in0=s1v,
                in1=attr4[:, :, 0:1].to_broadcast([P, 4, 16]),
                op=mybir.AluOpType.mult)
            # z_vs (zm) = v1[(u,m)->(m,u)] * s2
            nc.gpsimd.tensor_tensor(
                out=Z4v[:, :, 80:128].rearrange("p s (m u) -> p s m u", u=16),
                in0=src4[:, :, 16:64].rearrange("p s (u m) -> p s m u", m=3),
                in1=attr4[:, :, 0:1][:, :, :, None].to_broadcast([P, 4, 3, 16]),
                op=mybir.AluOpType.mult)
            # z_sv = s1 (x) v2
            nc.gpsimd.tensor_tensor(
                out=Z4v[:, :, 32:80].rearrange("p s (m u) -> p s m u", u=16),
                in0=s1v[:, :, None, :].to_broadcast([P, 4, 3, 16]),
                in1=attr4[:, :, 1:4][:, :, :, None].to_broadcast([P, 4, 3, 16]),
                op=mybir.AluOpType.mult)
            # pvv = v1 * v2 (u,m); zvv = sum_m (bf16 accumulate is fine here:
            # 3-term sum feeding a bf16 product path)
            pvv = sbB.tile([P, 192], F32, tag="pvv")
            nc.gpsimd.tensor_tensor(
                out=pvv[:].rearrange("p (s u m) -> p s u m", s=4, m=3),
                in0=src4[:, :, 16:64].rearrange("p s (u m) -> p s u m", m=3),
                in1=attr4[:, :, 1:4][:, :, None, :].to_broadcast([P, 4, 16, 3]),
                op=mybir.AluOpType.mult)
            with nc.allow_low_precision("zvv: 3-term bf16 sum within tolerance"):
                nc.vector.reduce_sum(
                    Z4v[:, :, 16:32][:, :, :, None],
                    pvv[:].rearrange("p (s u m) -> p s u m", s=4, m=3),
                    axis=mybir.AxisListType.X)

            for s in range(4):
                t_idx = g * 4 + s
                blk = tile_blk[t_idx]

                # ---- W2 GEMM. Host col layout: [0:512]=[sv|vs] (drained),
                # [512:1024]=[ss|vv] (kept in PSUM, read by DVE directly).
                psA = psW.tile([P, 512], F32, space="PSUM", tag="wps")
                for kk in range(2):
                    nc.tensor.matmul(psA[:], gT[kk][:, s * P:(s + 1) * P],
                                     w2_h[kk][:, 512:1024],
                                     start=(kk == 0), stop=(kk == 1))
                psB = psW.tile([P, 512], F32, space="PSUM", tag="wps")
                for kk in range(2):
                    nc.tensor.matmul(psB[:], gT[kk][:, s * P:(s + 1) * P],
                                     w2_h[kk][:, 0:512],
                                     start=(kk == 0), stop=(kk == 1))
                wbB = sbT.tile([P, 512], BF16, tag="wbB")
                nc.scalar.activation(wbB[:], psB[:],
                                     mybir.ActivationFunctionType.Copy)

                # ---- products -> t [128, 2048] bf16, cols = (c, u) u-inner
                # c = [ss_v 16 | vv_v 16 | sv (m,v) 48 | vs (m,v) 48]
                t_t = sbT.tile([P, 2048], BF16, tag="t_t")
                Zs = Z4[:, s * 128:(s + 1) * 128]
                # ss+vv from PSUM chunk A
                nc.vector.tensor_mul(
                    t_t[:, 0:512].rearrange("p (g v u) -> p g v u", g=2, u=16),
                    psA[:].rearrange("p (g v u) -> p g v u", g=2, u=16),
                    Zs[:, 0:32].rearrange("p (g u) -> p g u", g=2)[:, :, None, :]
                        .to_broadcast([P, 2, 16, 16]))
                # sv: t[(m,v,u)] = wbB_sv[(v,u)] * z_sv[(m,u)]  (m-outer: src0
                # streams 256 contiguous elements per m)
                nc.vector.tensor_mul(
                    t_t[:, 512:1280].rearrange("p (m v u) -> p m v u", v=16, u=16),
                    wbB[:, None, 0:256].rearrange("p x (v u) -> p x v u", u=16)
                        .to_broadcast([P, 3, 16, 16]),
                    Zs[:, 32:80].rearrange("p (m u) -> p m u", u=16)[:, :, None, :]
                        .to_broadcast([P, 3, 16, 16]))
                # vs: t[(m,v,u)] = wbB_vs[(v,u)] * z_vs[(m,u)]
                nc.vector.tensor_mul(
                    t_t[:, 1280:2048].rearrange("p (m v u) -> p m v u", v=16, u=16),
                    wbB[:, None, 256:512].rearrange("p x (v u) -> p x v u", u=16)
                        .to_broadcast([P, 3, 16, 16]),
                    Zs[:, 80:128].rearrange("p (m u) -> p m u", u=16)[:, :, None, :]
                        .to_broadcast([P, 3, 16, 16]))

                # ---- scatter: psO += Sel.T @ t  (4 x 512-col matmuls)
                if first[t_idx]:
                    sc_ps = psO.tile([P, 2048], F32, space="PSUM", tag="scps")
                    cur_blk = blk
                assert cur_blk == blk
                sel_s = sel4[:, s * P:(s + 1) * P]
                for c5 in range(4):
                    nc.tensor.matmul(sc_ps[:, c5 * 512:(c5 + 1) * 512],
                                     sel_s,
                                     t_t[:, c5 * 512:(c5 + 1) * 512],
                                     start=first[t_idx], stop=last[t_idx])

                if last[t_idx]:
                    # drain psO -> bf16, reduce over u, assemble, store
                    acc = sbO.tile([P, 2048], BF16, tag="acc")
                    nc.scalar.activation(acc[:, 0:1024], sc_ps[:, 0:1024],
                                         mybir.ActivationFunctionType.Copy)
                    nc.vector.tensor_copy(acc[:, 1024:2048], sc_ps[:, 1024:2048])
                    red = sbO.tile([P, 128], F32, tag="red")
                    nc.vector.reduce_sum(
                        red[:][:, :, None],
                        acc[:].rearrange("p (c u) -> p c u", u=16),
                        axis=mybir.AxisListType.X)
                    outb = sbO.tile([P, 64], F32, tag="outb")
                    nc.vector.tensor_add(outb[:, 0:16], red[:, 0:16], red[:, 16:32])
                    # sv/vs blocks are (m,v); output wants (v,m)
                    nc.vector.tensor_add(
                        outb[:, 16:64].rearrange("p (v m) -> p m v", m=3),
                        red[:, 32:80].rearrange("p (m v) -> p m v", v=16),
                        red[:, 80:128].rearrange("p (m v) -> p m v", v=16))
                    nc.sync.dma_start(out_d[blk * P:(blk + 1) * P, :], outb[:])

    nc.finalize()
    return nc


def _balance_blocks(edst):
    """Per core: bin-pack nodes into NBLK blocks of <=128 nodes, balancing
    per-block edge counts. Returns row[node] (position in the padded
    [NBLK*128] output) for every global node id."""
    row = np.empty(N_NODES, np.int64)
    deg_all = np.bincount(edst, minlength=N_NODES)
    for k in range(NCORES):
        nodes = np.arange(k * NPC, (k + 1) * NPC)
        deg = deg_all[nodes]
        order = np.argsort(-deg, kind="stable")
        sums = np.zeros(NBLK, np.int64)
        fill = np.zeros(NBLK, np.int64)
        for n in order:
            cand = np.where(fill < P)[0]
            b = cand[np.argmin(sums[cand])]
            row[nodes[n]] = b * P + fill[b]
            fill[b] += 1
            sums[b] += deg[n]
    return row


def _prep(inputs):
    """Host-side sharding: bucket edges by (core, node-block), pad to a
    uniform per-block tile grid, gather source features, build per-core
    input maps. Pure data movement + dtype casts (no value arithmetic)."""
    nf = np.ascontiguousarray(inputs["node_features"], dtype=np.float32)
    esrc = inputs["edge_src"].astype(np.int64)
    edst = inputs["edge_dst"].astype(np.int64)
    eattr = np.asarray(inputs["edge_attr"], dtype=np.float32)
    eemb = np.asarray(inputs["edge_embedding"], dtype=np.float32)

    row = _balance_blocks(edst)
    core = edst // NPC
    erow = row[edst]                      # padded row of the dst node
    blk = erow // P
    key = core * NBLK + blk
    order = np.argsort(key, kind="stable")
    key_s = key[order]

    counts = np.bincount(key_s, minlength=NCORES * NBLK).reshape(NCORES, NBLK)
    tiles_per_block = [int(math.ceil(max(counts[:, b].max(), 1) / P))
                       for b in range(NBLK)]
    while sum(tiles_per_block) % 4:
        tiles_per_block[-1] += 1
    T = sum(tiles_per_block)
    E = T * P
    NG = T // 4
    starts = np.concatenate([[0], np.cumsum([t * P for t in tiles_per_block])])

    import ml_dtypes
    bf16 = ml_dtypes.bfloat16

    in_maps = []
    boundaries = np.searchsorted(key_s, np.arange(NCORES * NBLK + 1))
    for k in range(NCORES):
        emb_full = np.zeros((E, 64), np.float32)
        src_full = np.zeros((E, 64), np.float32)
        attr_full = np.zeros((E, 4), np.float32)
        dst_full = np.full((E,), 999.0, np.float32)
        for b in range(NBLK):
            lo, hi = boundaries[k * NBLK + b], boundaries[k * NBLK + b + 1]
            n = hi - lo
            sl = order[lo:hi]
            o = int(starts[b])
            emb_full[o:o + n] = eemb[sl]
            src_full[o:o + n] = nf[esrc[sl]]
            attr_full[o:o + n] = eattr[sl]
            dst_full[o:o + n] = (erow[sl] - b * P).astype(np.float32)
        embT = np.ascontiguousarray(emb_full.T.astype(bf16))
        srcp = np.ascontiguousarray(
            src_full.reshape(NG, 4, P, 64).transpose(0, 2, 1, 3)
            .reshape(NG * P, 256).astype(bf16))
        attrp = np.ascontiguousarray(
            attr_full.reshape(NG, 4, P, 4).transpose(0, 2, 1, 3)
            .reshape(NG * P, 16))
        dstr = np.ascontiguousarray(
            dst_full.reshape(NG, 4, P).transpose(0, 2, 1)
            .reshape(NG * P, 4).astype(bf16))
        in_maps.append(dict(embT=embT, srcp=srcp, attrp=attrp, dstr=dstr))
    return tiles_per_block, in_maps, row


def _w2_scaled_perm():
    """W2 cols permuted to [sv|vs|ss|vv], each (v,u) u-inner, with path norm
    constants folded in. Returns (perm, scales)."""
    idx = np.arange(1024).reshape(4, MUL, MUL)  # (path, u, v): ss, vs, sv, vv
    blocks = [idx[2].T, idx[1].T, idx[0].T, idx[3].T]  # sv, vs, ss, vv as (v,u)
    perm = np.concatenate([b.reshape(-1) for b in blocks])
    scales = np.concatenate([
        np.full(256, ACGN, np.float32),   # sv
        np.full(256, ACGN, np.float32),   # vs
        np.full(256, AGN, np.float32),    # ss
        np.full(256, ACGN, np.float32),   # vv
    ])
    return perm, scales


def _full_maps(inputs):
    import ml_dtypes
    bf16 = ml_dtypes.bfloat16
    tiles_per_block, in_maps, row = _prep(inputs)
    perm, scales = _w2_scaled_perm()
    W1 = np.ascontiguousarray(np.asarray(inputs["W1"], np.float32).astype(bf16))
    b1 = np.ascontiguousarray(inputs["b1"], np.float32).reshape(HID, 1)
    W2p = np.ascontiguousarray(
        (np.asarray(inputs["W2"], np.float32)[:, perm] * scales[None, :])
        .astype(bf16))
    iota = np.tile(np.arange(P, dtype=np.float32)[None, :], (P, 1)).astype(bf16)
    assert not np.any(inputs["b2"]), "b2 != 0 unsupported by this build"
    for m in in_maps:
        m.update(w1=W1, b1=b1, w2=W2p, iota=iota)
    return tiles_per_block, in_maps, row


def kernel(**inputs):
    tiles_per_block, in_maps, row = _full_maps(inputs)
    key = tuple(tiles_per_block)
    if key not in _CACHE:
        _CACHE[key] = _build(tiles_per_block)
    nc = _CACHE[key]

    res = bass_utils.run_bass_kernel_spmd(nc, in_maps,
                                          core_ids=list(range(NCORES)))
    out = np.empty((N_NODES, 64), np.float32)
    for k in range(NCORES):
        nodes = np.arange(k * NPC, (k + 1) * NPC)
        out[nodes] = res.results[k]["out"][row[nodes]]
    return out
